# revision 1
# baseline (speedup 1.0000x reference)
"""Trainium2 Bass kernel for nn_LossFunction_40346922778857.

Computes: scatter-loss over x (256,128,768).
  x1 = x[::2], x2 = x[1::2]  (each (128,128,768))
  per half: within (D,D), between (D,D) scatter matrices, corr-normalized,
  loss = sum((w1-w2)^2) + sum((b1-b2)^2).

Strategy (data-parallel over b across 8 cores):
  within = (G - N * Xbar^T Xbar) / (B*N)   with G = X^T X over (B*N, D)
  between = N * (Xbar^T Xbar - B mean mean^T) / (B*N)
  Each core computes partial G (upper-triangle 128-row blocks, fp16 inputs,
  fp32 PSUM accumulation) for its 16 even + 16 odd b's.  Per-b row-sums S
  fall out of the same matmuls via 16 appended one-hot columns.
  Host sums the 8 partials and finishes the O(D^2) algebra.
"""

import numpy as np

P = 128          # partitions / rows per b
D = 768          # feature dim
NB = 16          # number of b's (tiles) per half per core
DA = D + NB      # augmented width (one-hot tile-index columns)
L = 4            # k-tiles per DMA quarter
NQ = NB // L     # quarters per half
NCORES = 8
NBLK = D // P    # 6 row blocks of G

_STATE = {}
LAST = {}


def _build():
    import concourse.tile as tile
    from concourse import bacc, mybir

    nc = bacc.Bacc("TRN2", target_bir_lowering=False, debug=False,
                   num_devices=NCORES)

    xins = [nc.dram_tensor(f"x{h}", [NQ, P, L * DA], mybir.dt.float16,
                           kind="ExternalInput").ap() for h in range(2)]
    outs = [nc.dram_tensor(f"o{h}", [D, DA], mybir.dt.float32,
                           kind="ExternalOutput").ap() for h in range(2)]

    with tile.TileContext(nc) as tc:
        with tc.tile_pool(name="xp", bufs=2 * NQ) as xp, \
             tc.tile_pool(name="pp", bufs=4, space="PSUM") as pp, \
             tc.tile_pool(name="op", bufs=3) as op:
            for h in range(2):
                xin, oout = xins[h], outs[h]
                q_tiles = []
                for q in range(NQ):
                    xt = xp.tile([P, L * DA], mybir.dt.float16, tag="xt",
                                 name=f"x{h}q{q}")
                    nc.sync.dma_start(out=xt[:], in_=xin[q])
                    q_tiles.append(xt)
                for i in range(NBLK):
                    w_all = DA - P * i
                    chunks = []
                    off = 0
                    while off < w_all:
                        w = min(512, w_all - off)
                        chunks.append((off, w))
                        off += w
                    pts = [pp.tile([P, 512], mybir.dt.float32, tag="ps",
                                   name=f"ps{h}b{i}c{ci}")
                           for ci in range(len(chunks))]
                    for t in range(NB):
                        q, l = divmod(t, L)
                        xt = q_tiles[q]
                        base = l * DA + P * i
                        lhsT = xt[:, base:base + P]
                        for (off, w), pt in zip(chunks, pts):
                            nc.tensor.matmul(pt[:, :w], lhsT,
                                             xt[:, base + off:base + off + w],
                                             start=(t == 0), stop=(t == NB - 1))
                    ot = op.tile([P, w_all], mybir.dt.float32, tag="ot",
                                 name=f"o{h}b{i}")
                    for (off, w), pt in zip(chunks, pts):
                        nc.vector.tensor_copy(ot[:, off:off + w], pt[:, :w])
                    nc.sync.dma_start(out=oout[P * i:P * (i + 1), P * i:DA],
                                      in_=ot[:])
    nc.compile()
    return nc


def _get_nc():
    if "nc" not in _STATE:
        _STATE["nc"] = _build()
    return _STATE["nc"]


def _prep_half(xh):
    """xh: (128, 128, 768) f32 for one half -> per-core list of (NQ,P,L*DA) f16."""
    out = []
    for c in range(NCORES):
        blk = xh[NB * c:NB * (c + 1)]                      # (16, 128, 768)
        arr = np.zeros((NB, P, DA), dtype=np.float16)
        arr[:, :, :D] = blk
        for j in range(NB):
            arr[j, :, D + j] = 1.0
        # (t=4q+l, p, f) -> (q, p, l*DA+f)
        out.append(np.ascontiguousarray(
            arr.reshape(NQ, L, P, DA).transpose(0, 2, 1, 3).reshape(NQ, P, L * DA)))
    return out


def kernel(x, label=None, genre_label=None, _trace=False):
    from concourse.bass_utils import run_bass_kernel_spmd

    nc = _get_nc()

    x = np.asarray(x, dtype=np.float32)
    halves = [_prep_half(x[0::2]), _prep_half(x[1::2])]
    in_maps = [{"x0": halves[0][c], "x1": halves[1][c]} for c in range(NCORES)]

    res = run_bass_kernel_spmd(nc, in_maps, list(range(NCORES)), trace=_trace)
    LAST["res"] = res

    B = x.shape[0] // 2          # 128 b's per half
    N = x.shape[1]               # 128 rows per b
    tol = B * N

    loss = 0.0
    for h in range(2):
        U = np.zeros((D, D), dtype=np.float64)
        S = np.zeros((B, D), dtype=np.float64)
        for c in range(NCORES):
            o = np.asarray(res.results[c][f"o{h}"], dtype=np.float64)
            for i in range(NBLK):
                r = slice(P * i, P * (i + 1))
                U[r, P * i:D] += o[r, P * i:D]
            S[NB * c:NB * (c + 1)] += o[:, D:DA].T
        G = np.zeros((D, D), dtype=np.float64)
        for i in range(NBLK):
            ri = slice(P * i, P * (i + 1))
            G[ri, ri] = U[ri, ri]
            for j in range(i + 1, NBLK):
                rj = slice(P * j, P * (j + 1))
                G[ri, rj] = U[ri, rj]
                G[rj, ri] = U[ri, rj].T
        xbar = S / N
        M = xbar.T @ xbar
        mean = xbar.mean(axis=0)
        within = (G - N * M) / tol
        between = N * (M - B * np.outer(mean, mean)) / tol
        w_h = within / np.sqrt(np.sum(np.diagonal(within) ** 2))
        b_h = between / np.sqrt(np.sum(np.diagonal(between) ** 2))
        if h == 0:
            w0, b0 = w_h, b_h
        else:
            loss = np.sum((w0 - w_h) ** 2) + np.sum((b0 - b_h) ** 2)
    return np.asarray(loss, dtype=np.float32)
